# revision 1
# baseline (speedup 1.0000x reference)
"""GCN message-passing kernel for 8 trn2 NeuronCores.

Math:  out = segment_sum(h[edge_src], edge_dst) @ W_post + b_post,
       h = data @ W_pre + b_pre.
By linearity:
       out[d] = (sum_{e: dst=d} data[src_e]) @ (W_pre @ W_post)
                + deg[d] * (b_pre @ W_post) + b_post

Sharding: dst-node shards of 12500 per core (fully independent — no
collectives).  Each core gathers raw 512B data rows for the edges landing in
its shard (dma_gather, int16 indices windowed by src range), segment-sums
them with one-hot matmuls on the TensorEngine (PSUM accumulation per
128-node dst block), applies the folded projection, and writes its output
shard transposed ([64, shard]); the host re-assembles.

Self-contained: only numpy + concourse imports; all shapes hardcoded.
"""

from contextlib import ExitStack

import numpy as np

import concourse.bacc as bacc
import concourse.mybir as mybir
import concourse.tile as tile
from concourse import library_config
from concourse.bass_utils import run_bass_kernel_spmd

F32 = mybir.dt.float32
I16 = mybir.dt.int16


class Cfg:
    N = 100000          # nodes
    DIN = 128           # input features
    DOUT = 64           # output features
    NC = 8              # cores
    SH = 12500          # dst nodes per core
    BS = 128            # dst block size
    NB = 98             # ceil(SH/BS) blocks per core
    NW = 4              # src windows
    WS = 25000          # window size (int16-safe)
    CU = 5              # uniform chunks per (block, window) cell
    G = 6               # blocks per gather group (6 acc psum banks + 2 out)


def _derived(cfg):
    NB, G = cfg.NB, cfg.G
    group_sizes = []
    b = 0
    while b < NB:
        group_sizes.append(min(G, NB - b))
        b += G
    slots_per_cell = cfg.CU * 128
    tot_slots = cfg.NB * cfg.NW * slots_per_cell
    return group_sizes, slots_per_cell, tot_slots


def preprocess(edge_src, edge_dst, cfg=Cfg):
    """Per-core gather-index / dst-local / degree arrays (pure index math)."""
    group_sizes, spc, tot_slots = _derived(cfg)
    src = np.asarray(edge_src).astype(np.int64)
    dst = np.asarray(edge_dst).astype(np.int64)

    core = dst // cfg.SH
    loc_node = dst - core * cfg.SH
    blk = loc_node // cfg.BS
    loc = loc_node - blk * cfg.BS
    win = src // cfg.WS
    widx = src - win * cfg.WS

    # cell id (core, blk, win) and slot position inside the padded cell
    cell = (core * cfg.NB + blk) * cfg.NW + win
    order = np.argsort(cell, kind="stable")
    cell_s = cell[order]
    counts = np.bincount(cell, minlength=cfg.NC * cfg.NB * cfg.NW)
    assert counts.max() <= spc, (counts.max(), spc)
    starts = np.zeros(cfg.NC * cfg.NB * cfg.NW, np.int64)
    starts[1:] = np.cumsum(counts)[:-1]
    rank = np.arange(len(src)) - starts[cell_s]

    # cell -> slot base inside its core's slot array, laid out gather-major:
    # for g in groups: for w in windows: for b in group: [CU*128 slots]
    cell_base = np.zeros((cfg.NB, cfg.NW), np.int64)
    gather_offsets = []   # (group, win) -> (slot_base, n_slots)
    off = 0
    b0 = 0
    for gs in group_sizes:
        for w in range(cfg.NW):
            gather_offsets.append((off, gs * spc))
            for bi in range(gs):
                cell_base[b0 + bi, w] = off + bi * spc
            off += gs * spc
        b0 += gs
    assert off == tot_slots

    slot = cell_base[blk[order], win[order]] + rank  # slot within core

    idx_all = np.zeros((cfg.NC, tot_slots), np.int16)
    loc_all = np.full((cfg.NC, tot_slots), -1.0, np.float32)
    core_s = core[order]
    idx_all[core_s, slot] = widx[order].astype(np.int16)
    loc_all[core_s, slot] = loc[order].astype(np.float32)

    # wrap into DMA layouts
    idx_dram = np.zeros((cfg.NC, 128, tot_slots // 16), np.int16)
    loc_dram = np.zeros((cfg.NC, 128, tot_slots // 128), np.float32)
    for sbase, n in gather_offsets:
        lin = idx_all[:, sbase:sbase + n]                      # [NC, n]
        wrapped = lin.reshape(cfg.NC, n // 16, 16).transpose(0, 2, 1)  # [NC,16,n/16]
        idx_dram[:, :, sbase // 16: (sbase + n) // 16] = np.tile(wrapped, (1, 8, 1))
        ll = loc_all[:, sbase:sbase + n]
        loc_dram[:, :, sbase // 128: (sbase + n) // 128] = (
            ll.reshape(cfg.NC, n // 128, 128).transpose(0, 2, 1))

    # local node ln sits at block ln//128, pos ln%128 -> flat index ln
    deg_dram = np.zeros((cfg.NC, 1, cfg.NB * 128), np.float32)
    degs = np.bincount(dst, minlength=cfg.N).astype(np.float32)
    for c in range(cfg.NC):
        deg_dram[c, 0, : cfg.SH] = degs[c * cfg.SH:(c + 1) * cfg.SH]

    return idx_dram, loc_dram, deg_dram, gather_offsets, group_sizes


def build_program(cfg=Cfg):
    group_sizes, spc, tot_slots = _derived(cfg)
    nc = bacc.Bacc("TRN2", target_bir_lowering=False, debug=True)

    data = nc.dram_tensor("data", [cfg.N, cfg.DIN], F32, kind="ExternalInput")
    idxs = nc.dram_tensor("idxs", [128, tot_slots // 16], I16, kind="ExternalInput")
    locs = nc.dram_tensor("locs", [128, tot_slots // 128], F32, kind="ExternalInput")
    deg = nc.dram_tensor("deg", [1, cfg.NB * 128], F32, kind="ExternalInput")
    iota_in = nc.dram_tensor("iota", [128, 128], F32, kind="ExternalInput")
    ident_in = nc.dram_tensor("ident", [128, 128], F32, kind="ExternalInput")
    wpre_in = nc.dram_tensor("wpre", [cfg.DIN, cfg.DOUT], F32, kind="ExternalInput")
    wpost_in = nc.dram_tensor("wpost", [cfg.DOUT, cfg.DOUT], F32, kind="ExternalInput")
    bpre_in = nc.dram_tensor("bpre", [cfg.DOUT, 1], F32, kind="ExternalInput")
    bpost_in = nc.dram_tensor("bpost", [1, cfg.DOUT], F32, kind="ExternalInput")
    out = nc.dram_tensor("out", [cfg.DOUT, cfg.NB * 128], F32, kind="ExternalOutput")

    with tile.TileContext(nc) as tc, ExitStack() as stk:
        nc.gpsimd.load_library(library_config.mlp)
        with (
            tc.tile_pool(name="consts", bufs=1) as cpool,
            tc.tile_pool(name="idxp", bufs=4) as idxp,
            tc.tile_pool(name="locp", bufs=4) as locp,
            tc.tile_pool(name="msgs", bufs=3) as msgsp,
            tc.tile_pool(name="oh", bufs=3) as ohp,
            tc.tile_pool(name="accsb", bufs=3) as accsbp,
            tc.tile_pool(name="outsb", bufs=2) as outsbp,
            tc.tile_pool(name="degp", bufs=2) as degp,
        ):
            # ---- constants & folded weights ----
            iota_sb = cpool.tile([128, 128], F32)
            ident_sb = cpool.tile([128, 128], F32)
            wpre_sb = cpool.tile([cfg.DIN, cfg.DOUT], F32)
            wpost_sb = cpool.tile([cfg.DOUT, cfg.DOUT], F32)
            bpre_sb = cpool.tile([cfg.DOUT, 1], F32)
            bpost_sb = cpool.tile([1, cfg.DOUT], F32)
            ones_sb = cpool.tile([1, 128], F32)
            nc.sync.dma_start(out=iota_sb[:], in_=iota_in[:])
            nc.sync.dma_start(out=ident_sb[:], in_=ident_in[:])
            nc.sync.dma_start(out=wpre_sb[:], in_=wpre_in[:])
            nc.sync.dma_start(out=wpost_sb[:], in_=wpost_in[:])
            nc.sync.dma_start(out=bpre_sb[:], in_=bpre_in[:])
            nc.sync.dma_start(out=bpost_sb[:], in_=bpost_in[:])
            nc.vector.memset(ones_sb[:], 1.0)

            with tc.tile_pool(name="pssetup", bufs=1, space="PSUM") as pssetup:
                wpreT_ps = pssetup.tile([cfg.DOUT, cfg.DIN], F32, tag="setup")
                nc.tensor.transpose(out=wpreT_ps[:], in_=wpre_sb[:],
                                    identity=ident_sb[:])
                wpreT_sb = cpool.tile([cfg.DOUT, cfg.DIN], F32)
                nc.vector.tensor_copy(wpreT_sb[:], wpreT_ps[:])

                wcomb_ps = pssetup.tile([cfg.DIN, cfg.DOUT], F32, tag="setup")
                nc.tensor.matmul(out=wcomb_ps[:], lhsT=wpreT_sb[:],
                                 rhs=wpost_sb[:], start=True, stop=True)
                wcomb_sb = cpool.tile([cfg.DIN, cfg.DOUT], F32)
                nc.vector.tensor_copy(wcomb_sb[:], wcomb_ps[:])

                bpw_ps = pssetup.tile([1, cfg.DOUT], F32, tag="setup")
                nc.tensor.matmul(out=bpw_ps[:], lhsT=bpre_sb[:], rhs=wpost_sb[:],
                                 start=True, stop=True)
                bpw_sb = cpool.tile([1, cfg.DOUT], F32)
                nc.vector.tensor_copy(bpw_sb[:], bpw_ps[:])

            psacc = stk.enter_context(
                tc.tile_pool(name="psacc", bufs=6, space="PSUM"))
            psout = stk.enter_context(
                tc.tile_pool(name="psout", bufs=2, space="PSUM"))
            # ---- main loop over gather groups ----
            # Window-sequential: each window's (msgs, onehot) pair is fully
            # consumed (all blocks' chunk-matmuls) before the next window's,
            # so only ~2 window tiles are live (double buffering); the G
            # per-block PSUM accumulators stay live across the 4 windows.
            off = 0      # slot offset
            b0 = 0       # first block of group
            for gs in group_sizes:
                n = gs * spc             # slots per gather here
                C = n // 128             # chunks per gather
                deg_t = degp.tile([1, gs * 128], F32)
                nc.sync.dma_start(out=deg_t[:],
                                  in_=deg[:, b0 * 128: (b0 + gs) * 128])
                accs = [psacc.tile([128, 128], F32, name=f"acc{b0}_{_i}", tag="acc")
                        for _i in range(gs)]
                for w in range(cfg.NW):
                    idx_t = idxp.tile([128, n // 16], I16)
                    nc.sync.dma_start(
                        out=idx_t[:], in_=idxs[:, off // 16: (off + n) // 16])
                    loc_t = locp.tile([128, C], F32)
                    nc.sync.dma_start(
                        out=loc_t[:], in_=locs[:, off // 128: (off + n) // 128])
                    m_t = msgsp.tile([128, C, cfg.DIN], F32)
                    nc.gpsimd.dma_gather(
                        m_t[:], data[w * cfg.WS: (w + 1) * cfg.WS, :], idx_t[:],
                        n, n, cfg.DIN, single_packet=False)
                    o_t = ohp.tile([128, C, 128], F32)
                    nc.vector.tensor_tensor(
                        out=o_t[:],
                        in0=loc_t[:].unsqueeze(2).broadcast_to([128, C, 128]),
                        in1=iota_sb[:].unsqueeze(1).broadcast_to([128, C, 128]),
                        op=mybir.AluOpType.is_equal)
                    for bi in range(gs):
                        for cu in range(cfg.CU):
                            ch = bi * cfg.CU + cu
                            nc.tensor.matmul(
                                out=accs[bi][:],
                                lhsT=m_t[:, ch, :],
                                rhs=o_t[:, ch, :],
                                start=(w == 0 and cu == 0),
                                stop=(w == cfg.NW - 1 and cu == cfg.CU - 1))
                    off += n

                out_t = outsbp.tile([cfg.DOUT, gs * 128], F32)
                for bi in range(gs):
                    acc_sb = accsbp.tile([128, 128], F32)
                    nc.scalar.copy(acc_sb[:], accs[bi][:])
                    outp = psout.tile([cfg.DOUT, 128], F32)
                    nc.tensor.matmul(out=outp[:], lhsT=wcomb_sb[:], rhs=acc_sb[:],
                                     start=True, stop=False)
                    nc.tensor.matmul(out=outp[:], lhsT=bpw_sb[:],
                                     rhs=deg_t[:, bi * 128:(bi + 1) * 128],
                                     start=False, stop=False)
                    nc.tensor.matmul(out=outp[:], lhsT=bpost_sb[:], rhs=ones_sb[:],
                                     start=False, stop=True)
                    nc.scalar.copy(out_t[:, bi * 128:(bi + 1) * 128], outp[:])
                nc.sync.dma_start(
                    out=out[:, b0 * 128:(b0 + gs) * 128], in_=out_t[:])
                b0 += gs
    nc.compile()
    return nc


_PROGRAM_CACHE = {}


def _get_program(cfg=Cfg):
    key = (cfg.N, cfg.CU, cfg.G)
    if key not in _PROGRAM_CACHE:
        _PROGRAM_CACHE[key] = build_program(cfg)
    return _PROGRAM_CACHE[key]


def make_in_maps(data, edge_src, edge_dst, W_pre, b_pre, W_post, b_post, cfg=Cfg):
    idx_dram, loc_dram, deg_dram, _, _ = preprocess(edge_src, edge_dst, cfg)
    data = np.ascontiguousarray(np.asarray(data, dtype=np.float32))
    iota = np.tile(np.arange(128, dtype=np.float32), (128, 1))
    ident = np.eye(128, dtype=np.float32)
    wpre = np.asarray(W_pre, dtype=np.float32)
    wpost = np.asarray(W_post, dtype=np.float32)
    bpre = np.asarray(b_pre, dtype=np.float32).reshape(cfg.DOUT, 1)
    bpost = np.asarray(b_post, dtype=np.float32).reshape(1, cfg.DOUT)
    in_maps = []
    for c in range(cfg.NC):
        in_maps.append({
            "data": data,
            "idxs": idx_dram[c],
            "locs": loc_dram[c],
            "deg": deg_dram[c],
            "iota": iota,
            "ident": ident,
            "wpre": wpre,
            "wpost": wpost,
            "bpre": bpre,
            "bpost": bpost,
        })
    return in_maps


def kernel(data, edge_src, edge_dst, W_pre, b_pre, W_post, b_post):
    cfg = Cfg
    nc = _get_program(cfg)
    in_maps = make_in_maps(data, edge_src, edge_dst, W_pre, b_pre, W_post,
                           b_post, cfg)
    res = run_bass_kernel_spmd(nc, in_maps, list(range(cfg.NC)), trace=False)
    out = np.empty((cfg.N, cfg.DOUT), np.float32)
    for c in range(cfg.NC):
        out[c * cfg.SH:(c + 1) * cfg.SH, :] = res.results[c]["out"][:, :cfg.SH].T
    return out



# revision 18
# speedup vs baseline: 1.0064x; 1.0064x over previous
"""GCN message-passing kernel for 8 trn2 NeuronCores.

Math:  out = segment_sum(h[edge_src], edge_dst) @ W_post + b_post,
       h = data @ W_pre + b_pre.
By linearity:
       out[d] = (sum_{e: dst=d} data[src_e]) @ (W_pre @ W_post)
                + deg[d] * (b_pre @ W_post) + b_post

Sharding: dst nodes are assigned to 784 bins of <=128 nodes (8 cores x 98
blocks) by a degree-balanced snake deal, so every core owns ~12500 nodes.
Each core gathers raw 512B data rows for the edges landing in its bins
(gpsimd dma_gather, int16 indices against one of 4 overlapping 32768-row
source windows), converts them to bf16 (Activation engine), segment-sums
them with one-hot bf16 matmuls on the TensorEngine (PSUM accumulation per
128-node dst block), applies the folded bf16 projection plus a rank-2 bias
term, and writes its output block-transposed ([64, 12544]); the host
scatters rows back to node order.

Window balancing: the 4 gather windows overlap (base + 32768 > next base),
so ~30% of edges may use either of two windows. A prefix-flow water-fill
per bin equalizes every (block, window) cell to <=512 edges, which makes a
uniform 4-chunks-per-cell program layout possible with only ~0.3% padding
(200832 slots/core vs 250880 for fixed 5-chunk cells).

Self-contained: only numpy + concourse imports; all shapes hardcoded.
"""

from contextlib import ExitStack

import numpy as np

import concourse.bacc as bacc
import concourse.mybir as mybir
import concourse.tile as tile
from concourse import library_config
from concourse.bass_utils import run_bass_kernel_spmd

F32 = mybir.dt.float32
BF16 = mybir.dt.bfloat16
I16 = mybir.dt.int16
NP_BF16 = mybir.dt.np(mybir.dt.bfloat16)


class Cfg:
    N = 100000          # nodes
    DIN = 128           # input features
    DOUT = 64           # output features
    NC = 8              # cores
    BS = 128            # dst block (bin) size
    NB = 98             # bins per core
    NW = 4              # src windows
    G = 6               # blocks per gather group (6 acc psum banks + 2 out)
    WBASE = (0, 22411, 44822, 67233)   # window bases (32768-row windows)


def _groups(cfg):
    sizes = []
    b = 0
    while b < cfg.NB:
        sizes.append(min(cfg.G, cfg.NB - b))
        b += cfg.G
    return sizes


def preprocess(edge_src, edge_dst, cfg=Cfg):
    """Node->bin assignment, window balancing, gather index layout."""
    src = np.asarray(edge_src).astype(np.int64)
    dst = np.asarray(edge_dst).astype(np.int64)
    E = len(src)
    N, NC, NB, NW = cfg.N, cfg.NC, cfg.NB, cfg.NW
    NBINS = NC * NB
    WBASE = np.asarray(cfg.WBASE)
    WTOP = WBASE + 32767

    # --- node -> bin: snake deal by total in-degree ---
    deg = np.bincount(dst, minlength=N)
    order = np.argsort(-deg, kind="stable")
    node_bin = np.empty(N, np.int64)
    nfull = (N + NBINS - 1) // NBINS
    for r in range(nfull):
        seg = order[r * NBINS:(r + 1) * NBINS]
        idxs = np.arange(len(seg))
        node_bin[seg] = idxs if r % 2 == 0 else (NBINS - 1 - idxs)
    # position within bin
    o2 = np.argsort(node_bin, kind="stable")
    starts = np.searchsorted(node_bin[o2], np.arange(NBINS))
    pos = np.empty(N, np.int64)
    pos[o2] = np.arange(N) - starts[node_bin[o2]]
    assert pos.max() < cfg.BS
    node_core = node_bin // NB
    node_slot = (node_bin % NB) * cfg.BS + pos

    # --- window assignment with prefix-flow balancing ---
    win = np.searchsorted(WBASE, src, side="right") - 1
    in_zone = np.full(E, -1, np.int64)
    for k in range(NW - 1):
        z = (src >= WBASE[k + 1]) & (src <= WTOP[k])
        in_zone[z] = k
        mid = (WBASE[k + 1] + WTOP[k]) // 2
        win[z] = np.where(src[z] <= mid, k, k + 1)

    ebin = node_bin[dst]
    cell = np.zeros((NBINS, NW), np.int64)
    np.add.at(cell, (ebin, win), 1)
    deg_bin = cell.sum(1)
    q, r4 = np.divmod(deg_bin, 4)
    d = np.stack([q + (r4 > 0), q + (r4 > 1), q + (r4 > 2), q], axis=1)
    F = np.cumsum(cell - d, axis=1)[:, :3]
    for k in range(NW - 1):
        f = F[:, k]
        for direction in (0, 1):
            if direction == 0:
                need = np.maximum(f, 0)
                mask = (in_zone == k) & (win == k)
            else:
                need = np.maximum(-f, 0)
                mask = (in_zone == k) & (win == k + 1)
            if need.sum() == 0:
                continue
            idx = np.where(mask)[0]
            b = ebin[idx]
            o = np.argsort(b, kind="stable")
            idx, b = idx[o], b[o]
            st = np.searchsorted(b, np.arange(NBINS))
            rank = np.arange(len(b)) - st[b]
            flip = idx[rank < need[b]]
            win[flip] = (k + 1) if direction == 0 else k

    cell = np.zeros((NBINS, NW), np.int64)
    np.add.at(cell, (ebin, win), 1)

    # --- uniform chunk table: max over cores, ceil to 128 ---
    mx = cell.reshape(NC, NB, NW).max(axis=0)
    chunks = -(-mx // 128)          # [NB, NW]
    chunks = np.maximum(chunks, 1)

    # --- slot layout: group-major, per (group, window) gather calls ---
    group_sizes = _groups(cfg)
    cell_base = np.zeros((NB, NW), np.int64)
    gather_calls = []               # (slot_base, n_slots) per (g, w)
    off = 0
    b0 = 0
    for gs in group_sizes:
        for w in range(NW):
            base = off
            for bi in range(gs):
                cell_base[b0 + bi, w] = off
                off += chunks[b0 + bi, w] * 128
            gather_calls.append((base, off - base))
        b0 += gs
    tot_slots = off
    assert tot_slots % 128 == 0

    # --- per-core slot arrays ---
    core_e = ebin // NB
    blk_e = ebin % NB
    widx = src - WBASE[win]
    assert widx.min() >= 0 and widx.max() < 32768
    loc_e = pos[dst]

    cell_id = (core_e * NB + blk_e) * NW + win
    o3 = np.argsort(cell_id, kind="stable")
    cid_s = cell_id[o3]
    counts = np.bincount(cell_id, minlength=NC * NB * NW)
    cap = (np.tile(chunks[None], (NC, 1, 1)).reshape(-1) * 128)
    assert (counts <= cap).all(), (counts.max(), "cell overflow")
    st = np.zeros(NC * NB * NW, np.int64)
    st[1:] = np.cumsum(counts)[:-1]
    rank = np.arange(E) - st[cid_s]
    slot = cell_base[blk_e[o3], win[o3]] + rank

    idx_all = np.zeros((NC, tot_slots), np.int16)
    loc_all = np.full((NC, tot_slots), -1.0, np.float32)
    idx_all[core_e[o3], slot] = widx[o3].astype(np.int16)
    loc_all[core_e[o3], slot] = loc_e[o3].astype(np.float32)

    # --- DMA wrap layouts ---
    idx_dram = np.zeros((NC, 128, tot_slots // 16), np.int16)
    loc_dram = np.zeros((NC, 128, tot_slots // 128), np.float32)
    for sbase, n in gather_calls:
        lin = idx_all[:, sbase:sbase + n]
        wrapped = lin.reshape(NC, n // 16, 16).transpose(0, 2, 1)
        idx_dram[:, :, sbase // 16:(sbase + n) // 16] = np.tile(wrapped, (1, 8, 1))
        ll = loc_all[:, sbase:sbase + n]
        loc_dram[:, :, sbase // 128:(sbase + n) // 128] = (
            ll.reshape(NC, n // 128, 128).transpose(0, 2, 1))

    # row 0: node degree; row 1: ones (rank-2 bias matmul rhs)
    deg_dram = np.ones((NC, 2, NB * 128), NP_BF16)
    flat = node_core * (NB * 128) + node_slot
    dg = np.zeros(NC * NB * 128, np.float32)
    dg[flat] = deg.astype(np.float32)
    deg_dram[:, 0, :] = dg.reshape(NC, NB * 128).astype(NP_BF16)

    chunk_key = tuple(map(int, chunks.reshape(-1)))
    return (idx_dram, loc_dram, deg_dram, chunk_key,
            node_core.astype(np.int64), node_slot.astype(np.int64), tot_slots)


def build_program(chunk_key, cfg=Cfg):
    NB, NW = cfg.NB, cfg.NW
    chunks = np.asarray(chunk_key, np.int64).reshape(NB, NW)
    group_sizes = _groups(cfg)
    tot_slots = int(chunks.sum()) * 128

    nc = bacc.Bacc("TRN2", target_bir_lowering=False, debug=True)

    data = nc.dram_tensor("data", [cfg.N, cfg.DIN], F32, kind="ExternalInput")
    idxs = nc.dram_tensor("idxs", [128, tot_slots // 16], I16, kind="ExternalInput")
    locs = nc.dram_tensor("locs", [128, tot_slots // 128], F32, kind="ExternalInput")
    deg = nc.dram_tensor("deg", [2, NB * 128], BF16, kind="ExternalInput")
    iota_in = nc.dram_tensor("iota", [128, 128], F32, kind="ExternalInput")
    ident_in = nc.dram_tensor("ident", [128, 128], F32, kind="ExternalInput")
    wpre_in = nc.dram_tensor("wpre", [cfg.DIN, cfg.DOUT], F32, kind="ExternalInput")
    wpost_in = nc.dram_tensor("wpost", [cfg.DOUT, cfg.DOUT], F32, kind="ExternalInput")
    bpre_in = nc.dram_tensor("bpre", [cfg.DOUT, 1], F32, kind="ExternalInput")
    bposth_in = nc.dram_tensor("bposth", [1, cfg.DOUT], BF16, kind="ExternalInput")
    out = nc.dram_tensor("out", [cfg.DOUT, NB * 128], BF16, kind="ExternalOutput")

    with tile.TileContext(nc) as tc, ExitStack() as stk:
        nc.gpsimd.load_library(library_config.mlp)
        with (
            tc.tile_pool(name="consts", bufs=1) as cpool,
            tc.tile_pool(name="msgs", bufs=3) as msgsp,
            tc.tile_pool(name="msgsb", bufs=3) as msgsbp,
            tc.tile_pool(name="oh", bufs=3) as ohp,
            tc.tile_pool(name="accsb", bufs=3) as accsbp,
            tc.tile_pool(name="outsb", bufs=2) as outsbp,
        ):
            # ---- constants & folded weights ----
            iota_sb = cpool.tile([128, 128], F32)
            ident_sb = cpool.tile([128, 128], F32)
            wpre_sb = cpool.tile([cfg.DIN, cfg.DOUT], F32)
            wpost_sb = cpool.tile([cfg.DOUT, cfg.DOUT], F32)
            bpre_sb = cpool.tile([cfg.DOUT, 1], F32)
            nc.sync.dma_start(out=iota_sb[:], in_=iota_in[:])
            nc.sync.dma_start(out=ident_sb[:], in_=ident_in[:])
            nc.sync.dma_start(out=wpre_sb[:], in_=wpre_in[:])
            nc.sync.dma_start(out=wpost_sb[:], in_=wpost_in[:])
            nc.sync.dma_start(out=bpre_sb[:], in_=bpre_in[:])

            # whole-program index/loc/deg tables: one full-rate DMA each
            idx_all = cpool.tile([128, tot_slots // 16], I16)
            nc.sync.dma_start(out=idx_all[:], in_=idxs[:])
            loc_all = cpool.tile([128, tot_slots // 128], F32)
            nc.sync.dma_start(out=loc_all[:], in_=locs[:])
            deg_all = cpool.tile([2, NB * 128], BF16)
            nc.sync.dma_start(out=deg_all[:], in_=deg[:])

            wcomb_sb = cpool.tile([cfg.DIN, cfg.DOUT], BF16)
            bias2_sb = cpool.tile([2, cfg.DOUT], BF16)
            with tc.tile_pool(name="pssetup", bufs=1, space="PSUM") as pssetup:
                wpreT_ps = pssetup.tile([cfg.DOUT, cfg.DIN], F32, tag="setup")
                nc.tensor.transpose(out=wpreT_ps[:], in_=wpre_sb[:],
                                    identity=ident_sb[:])
                wpreT_sb = cpool.tile([cfg.DOUT, cfg.DIN], F32)
                nc.vector.tensor_copy(wpreT_sb[:], wpreT_ps[:])

                wcomb_ps = pssetup.tile([cfg.DIN, cfg.DOUT], F32, tag="setup")
                nc.tensor.matmul(out=wcomb_ps[:], lhsT=wpreT_sb[:],
                                 rhs=wpost_sb[:], start=True, stop=True)
                nc.vector.tensor_copy(wcomb_sb[:], wcomb_ps[:])

                bpw_ps = pssetup.tile([1, cfg.DOUT], F32, tag="setup")
                nc.tensor.matmul(out=bpw_ps[:], lhsT=bpre_sb[:], rhs=wpost_sb[:],
                                 start=True, stop=True)
                nc.vector.tensor_copy(bias2_sb[0:1, :], bpw_ps[:])
                # engine writes may not start at partition 1; DMA can
                nc.sync.dma_start(out=bias2_sb[1:2, :], in_=bposth_in[:])

            psacc = stk.enter_context(
                tc.tile_pool(name="psacc", bufs=6, space="PSUM"))
            psout = stk.enter_context(
                tc.tile_pool(name="psout", bufs=2, space="PSUM"))

            # per-block first/last (w, cu) for psum start/stop flags
            first_wc = {}
            last_wc = {}
            for b in range(NB):
                pres = [(w, cu) for w in range(NW) for cu in range(chunks[b, w])]
                first_wc[b] = pres[0]
                last_wc[b] = pres[-1]

            # ---- main loop over gather groups ----
            off = 0
            b0 = 0
            for gs in group_sizes:
                accs = [psacc.tile([128, 128], F32, name=f"acc{b0}_{i}", tag="acc")
                        for i in range(gs)]
                for w in range(NW):
                    n = int(chunks[b0:b0 + gs, w].sum()) * 128
                    C = n // 128
                    m_t = msgsp.tile([128, C, cfg.DIN], F32)
                    wb = cfg.WBASE[w]
                    nc.gpsimd.dma_gather(
                        m_t[:], data[wb:min(cfg.N, wb + 32768), :],
                        idx_all[:, off // 16:(off + n) // 16],
                        n, n, cfg.DIN, single_packet=False)
                    mb_t = msgsbp.tile([128, C, cfg.DIN], BF16)
                    nc.scalar.copy(mb_t[:], m_t[:])
                    loc_c = loc_all[:, off // 128:(off + n) // 128]
                    o_t = ohp.tile([128, C, 128], BF16)
                    nc.vector.tensor_tensor(
                        out=o_t[:],
                        in0=loc_c.unsqueeze(2).broadcast_to([128, C, 128]),
                        in1=iota_sb[:].unsqueeze(1).broadcast_to([128, C, 128]),
                        op=mybir.AluOpType.is_equal)
                    ch = 0
                    for bi in range(gs):
                        b = b0 + bi
                        for cu in range(chunks[b, w]):
                            nc.tensor.matmul(
                                out=accs[bi][:],
                                lhsT=mb_t[:, ch, :],
                                rhs=o_t[:, ch, :],
                                start=(w, cu) == first_wc[b],
                                stop=(w, cu) == last_wc[b])
                            ch += 1
                    off += n

                out_t = outsbp.tile([cfg.DOUT, gs * 128], BF16)
                for bi in range(gs):
                    acc_sb = accsbp.tile([128, 128], BF16)
                    nc.vector.tensor_copy(acc_sb[:], accs[bi][:])
                    outp = psout.tile([cfg.DOUT, 128], F32)
                    nc.tensor.matmul(out=outp[:], lhsT=wcomb_sb[:], rhs=acc_sb[:],
                                     start=True, stop=False)
                    nc.tensor.matmul(out=outp[:], lhsT=bias2_sb[:],
                                     rhs=deg_all[:, (b0 + bi) * 128:
                                                 (b0 + bi + 1) * 128],
                                     start=False, stop=True)
                    nc.scalar.copy(out_t[:, bi * 128:(bi + 1) * 128], outp[:])
                nc.sync.dma_start(
                    out=out[:, b0 * 128:(b0 + gs) * 128], in_=out_t[:])
                b0 += gs
    nc.compile()
    return nc


_PROGRAM_CACHE = {}


def _get_program(chunk_key, cfg=Cfg):
    key = (cfg.N, cfg.G, chunk_key)
    if key not in _PROGRAM_CACHE:
        _PROGRAM_CACHE[key] = build_program(chunk_key, cfg)
    return _PROGRAM_CACHE[key]


def make_in_maps(data, edge_src, edge_dst, W_pre, b_pre, W_post, b_post, cfg=Cfg):
    (idx_dram, loc_dram, deg_dram, chunk_key, node_core, node_slot,
     tot_slots) = preprocess(edge_src, edge_dst, cfg)
    data = np.ascontiguousarray(np.asarray(data, dtype=np.float32))
    iota = np.tile(np.arange(128, dtype=np.float32), (128, 1))
    ident = np.eye(128, dtype=np.float32)
    wpre = np.asarray(W_pre, dtype=np.float32)
    wpost = np.asarray(W_post, dtype=np.float32)
    bpre = np.asarray(b_pre, dtype=np.float32).reshape(cfg.DOUT, 1)
    bpost = np.asarray(b_post, dtype=np.float32).reshape(1, cfg.DOUT)
    in_maps = []
    for c in range(cfg.NC):
        in_maps.append({
            "data": data,
            "idxs": idx_dram[c],
            "locs": loc_dram[c],
            "deg": deg_dram[c],
            "iota": iota,
            "ident": ident,
            "wpre": wpre,
            "wpost": wpost,
            "bpre": bpre,
            "bpost": bpost,
            "bposth": bpost.astype(NP_BF16),
        })
    return in_maps, chunk_key, node_core, node_slot


def assemble(outs, node_core, node_slot, cfg=Cfg):
    """outs: list of per-core [DOUT, NB*128] arrays -> [N, DOUT]."""
    stacked = np.stack([np.asarray(o) for o in outs])  # [NC, DOUT, NB*128]
    return stacked[node_core, :, node_slot].astype(np.float32)


def kernel(data, edge_src, edge_dst, W_pre, b_pre, W_post, b_post):
    cfg = Cfg
    in_maps, chunk_key, node_core, node_slot = make_in_maps(
        data, edge_src, edge_dst, W_pre, b_pre, W_post, b_post, cfg)
    nc = _get_program(chunk_key, cfg)
    res = run_bass_kernel_spmd(nc, in_maps, list(range(cfg.NC)), trace=False)
    return assemble([res.results[c]["out"] for c in range(cfg.NC)],
                    node_core, node_slot, cfg)


# revision 25
# speedup vs baseline: 1.0259x; 1.0193x over previous
"""GCN message-passing kernel for 8 trn2 NeuronCores.

Math:  out = segment_sum(h[edge_src], edge_dst) @ W_post + b_post,
       h = data @ W_pre + b_pre.
By linearity:
       out[d] = (sum_{e: dst=d} data[src_e]) @ (W_pre @ W_post)
                + deg[d] * (b_pre @ W_post) + b_post

Sharding: dst nodes are assigned to 784 bins of <=128 nodes (8 cores x 98
blocks) by a degree-balanced snake deal, so every core owns ~12500 nodes.
Each core gathers raw 512B data rows for the edges landing in its bins
(gpsimd dma_gather, int16 indices against one of 4 overlapping 32768-row
source windows), converts them to bf16 (Activation engine), segment-sums
them with one-hot bf16 matmuls on the TensorEngine (PSUM accumulation per
128-node dst block), applies the folded bf16 projection plus a rank-2 bias
term, and writes its output block-transposed ([64, 12544]); the host
scatters rows back to node order.

Window balancing: the 4 gather windows overlap (base + 32768 > next base),
so ~30% of edges may use either of two windows. A prefix-flow water-fill
per bin equalizes every (block, window) cell to <=512 edges, which makes a
uniform 4-chunks-per-cell program layout possible with only ~0.3% padding
(200832 slots/core vs 250880 for fixed 5-chunk cells).

Self-contained: only numpy + concourse imports; all shapes hardcoded.
"""

from contextlib import ExitStack

import numpy as np

import concourse.bacc as bacc
import concourse.mybir as mybir
import concourse.tile as tile
from concourse import library_config
from concourse.bass_utils import run_bass_kernel_spmd

F32 = mybir.dt.float32
BF16 = mybir.dt.bfloat16
I16 = mybir.dt.int16
NP_BF16 = mybir.dt.np(mybir.dt.bfloat16)


class Cfg:
    N = 100000          # nodes
    DIN = 128           # input features
    DOUT = 64           # output features
    NC = 8              # cores
    BS = 128            # dst block (bin) size
    NB = 98             # bins per core
    NW = 4              # src windows
    G = 6               # blocks per gather group (6 acc psum banks + 2 out)
    WBASE = (0, 22411, 44822, 67233)   # window bases (32768-row windows)


def _groups(cfg):
    sizes = []
    b = 0
    while b < cfg.NB:
        sizes.append(min(cfg.G, cfg.NB - b))
        b += cfg.G
    return sizes


def preprocess(edge_src, edge_dst, cfg=Cfg):
    """Node->bin assignment, window balancing, gather index layout."""
    src = np.asarray(edge_src).astype(np.int64)
    dst = np.asarray(edge_dst).astype(np.int64)
    E = len(src)
    N, NC, NB, NW = cfg.N, cfg.NC, cfg.NB, cfg.NW
    NBINS = NC * NB
    WBASE = np.asarray(cfg.WBASE)
    WTOP = WBASE + 32767

    # --- node -> bin: snake deal by total in-degree ---
    deg = np.bincount(dst, minlength=N)
    order = np.argsort(-deg, kind="stable")
    node_bin = np.empty(N, np.int64)
    nfull = (N + NBINS - 1) // NBINS
    for r in range(nfull):
        seg = order[r * NBINS:(r + 1) * NBINS]
        idxs = np.arange(len(seg))
        node_bin[seg] = idxs if r % 2 == 0 else (NBINS - 1 - idxs)
    # position within bin
    o2 = np.argsort(node_bin, kind="stable")
    starts = np.searchsorted(node_bin[o2], np.arange(NBINS))
    pos = np.empty(N, np.int64)
    pos[o2] = np.arange(N) - starts[node_bin[o2]]
    assert pos.max() < cfg.BS
    node_core = node_bin // NB
    node_slot = (node_bin % NB) * cfg.BS + pos

    # --- window assignment with prefix-flow balancing ---
    win = np.searchsorted(WBASE, src, side="right") - 1
    in_zone = np.full(E, -1, np.int64)
    for k in range(NW - 1):
        z = (src >= WBASE[k + 1]) & (src <= WTOP[k])
        in_zone[z] = k
        mid = (WBASE[k + 1] + WTOP[k]) // 2
        win[z] = np.where(src[z] <= mid, k, k + 1)

    ebin = node_bin[dst]
    cell = np.zeros((NBINS, NW), np.int64)
    np.add.at(cell, (ebin, win), 1)
    deg_bin = cell.sum(1)
    q, r4 = np.divmod(deg_bin, 4)
    d = np.stack([q + (r4 > 0), q + (r4 > 1), q + (r4 > 2), q], axis=1)
    F = np.cumsum(cell - d, axis=1)[:, :3]
    for k in range(NW - 1):
        f = F[:, k]
        for direction in (0, 1):
            if direction == 0:
                need = np.maximum(f, 0)
                mask = (in_zone == k) & (win == k)
            else:
                need = np.maximum(-f, 0)
                mask = (in_zone == k) & (win == k + 1)
            if need.sum() == 0:
                continue
            idx = np.where(mask)[0]
            b = ebin[idx]
            o = np.argsort(b, kind="stable")
            idx, b = idx[o], b[o]
            st = np.searchsorted(b, np.arange(NBINS))
            rank = np.arange(len(b)) - st[b]
            flip = idx[rank < need[b]]
            win[flip] = (k + 1) if direction == 0 else k

    cell = np.zeros((NBINS, NW), np.int64)
    np.add.at(cell, (ebin, win), 1)

    # --- uniform chunk table: max over cores, ceil to 128 ---
    mx = cell.reshape(NC, NB, NW).max(axis=0)
    chunks = -(-mx // 128)          # [NB, NW]
    chunks = np.maximum(chunks, 1)

    # --- slot layout: group-major, per (group, window) gather calls ---
    group_sizes = _groups(cfg)
    cell_base = np.zeros((NB, NW), np.int64)
    gather_calls = []               # (slot_base, n_slots) per (g, w)
    off = 0
    b0 = 0
    for gs in group_sizes:
        for w in range(NW):
            base = off
            for bi in range(gs):
                cell_base[b0 + bi, w] = off
                off += chunks[b0 + bi, w] * 128
            gather_calls.append((base, off - base))
        b0 += gs
    tot_slots = off
    assert tot_slots % 128 == 0

    # --- per-core slot arrays ---
    core_e = ebin // NB
    blk_e = ebin % NB
    widx = src - WBASE[win]
    assert widx.min() >= 0 and widx.max() < 32768
    loc_e = pos[dst]

    cell_id = (core_e * NB + blk_e) * NW + win
    o3 = np.argsort(cell_id, kind="stable")
    cid_s = cell_id[o3]
    counts = np.bincount(cell_id, minlength=NC * NB * NW)
    cap = (np.tile(chunks[None], (NC, 1, 1)).reshape(-1) * 128)
    assert (counts <= cap).all(), (counts.max(), "cell overflow")
    st = np.zeros(NC * NB * NW, np.int64)
    st[1:] = np.cumsum(counts)[:-1]
    rank = np.arange(E) - st[cid_s]
    slot = cell_base[blk_e[o3], win[o3]] + rank

    idx_all = np.zeros((NC, tot_slots), np.int16)
    loc_all = np.full((NC, tot_slots), -1.0, np.float32)
    idx_all[core_e[o3], slot] = widx[o3].astype(np.int16)
    loc_all[core_e[o3], slot] = loc_e[o3].astype(np.float32)

    # --- DMA wrap layouts ---
    idx_dram = np.zeros((NC, 128, tot_slots // 16), np.int16)
    loc_dram = np.zeros((NC, 128, tot_slots // 128), NP_BF16)
    for sbase, n in gather_calls:
        lin = idx_all[:, sbase:sbase + n]
        wrapped = lin.reshape(NC, n // 16, 16).transpose(0, 2, 1)
        idx_dram[:, :, sbase // 16:(sbase + n) // 16] = np.tile(wrapped, (1, 8, 1))
        ll = loc_all[:, sbase:sbase + n]
        loc_dram[:, :, sbase // 128:(sbase + n) // 128] = (
            ll.reshape(NC, n // 128, 128).transpose(0, 2, 1).astype(NP_BF16))

    # row 0: node degree; row 1: ones (rank-2 bias matmul rhs)
    deg_dram = np.ones((NC, 2, NB * 128), NP_BF16)
    flat = node_core * (NB * 128) + node_slot
    dg = np.zeros(NC * NB * 128, np.float32)
    dg[flat] = deg.astype(np.float32)
    deg_dram[:, 0, :] = dg.reshape(NC, NB * 128).astype(NP_BF16)

    chunk_key = tuple(map(int, chunks.reshape(-1)))
    return (idx_dram, loc_dram, deg_dram, chunk_key,
            node_core.astype(np.int64), node_slot.astype(np.int64), tot_slots)


def build_program(chunk_key, cfg=Cfg):
    NB, NW = cfg.NB, cfg.NW
    chunks = np.asarray(chunk_key, np.int64).reshape(NB, NW)
    group_sizes = _groups(cfg)
    tot_slots = int(chunks.sum()) * 128

    nc = bacc.Bacc("TRN2", target_bir_lowering=False, debug=True)

    data = nc.dram_tensor("data", [cfg.N, cfg.DIN], F32, kind="ExternalInput")
    idxs = nc.dram_tensor("idxs", [128, tot_slots // 16], I16, kind="ExternalInput")
    locs = nc.dram_tensor("locs", [128, tot_slots // 128], BF16, kind="ExternalInput")
    deg = nc.dram_tensor("deg", [2, NB * 128], BF16, kind="ExternalInput")
    iota_in = nc.dram_tensor("iota", [128, 128], BF16, kind="ExternalInput")
    ident_in = nc.dram_tensor("ident", [128, 128], F32, kind="ExternalInput")
    wpre_in = nc.dram_tensor("wpre", [cfg.DIN, cfg.DOUT], F32, kind="ExternalInput")
    wpost_in = nc.dram_tensor("wpost", [cfg.DOUT, cfg.DOUT], F32, kind="ExternalInput")
    bpre_in = nc.dram_tensor("bpre", [cfg.DOUT, 1], F32, kind="ExternalInput")
    bposth_in = nc.dram_tensor("bposth", [1, cfg.DOUT], BF16, kind="ExternalInput")
    out = nc.dram_tensor("out", [cfg.DOUT, NB * 128], BF16, kind="ExternalOutput")

    # slots covered by the first gather group: its idx/loc load first so
    # gathering starts while the rest of the tables stream in behind it
    head_slots = int(chunks[0:group_sizes[0], :].sum()) * 128

    with tile.TileContext(nc) as tc, ExitStack() as stk:
        nc.gpsimd.load_library(library_config.mlp)
        with (
            tc.tile_pool(name="consts", bufs=1) as cpool,
            tc.tile_pool(name="msgs", bufs=3) as msgsp,
            tc.tile_pool(name="msgsb", bufs=3) as msgsbp,
            tc.tile_pool(name="oh", bufs=3) as ohp,
            tc.tile_pool(name="accsb", bufs=3) as accsbp,
            tc.tile_pool(name="outsb", bufs=2) as outsbp,
        ):
            # ---- index/loc tables, head first ----
            iota_sb = cpool.tile([128, 128], BF16)
            idx_head = cpool.tile([128, head_slots // 16], I16)
            nc.sync.dma_start(out=idx_head[:], in_=idxs[:, :head_slots // 16])
            loc_head = cpool.tile([128, head_slots // 128], BF16)
            nc.sync.dma_start(out=loc_head[:], in_=locs[:, :head_slots // 128])
            nc.sync.dma_start(out=iota_sb[:], in_=iota_in[:])
            idx_rest = cpool.tile([128, (tot_slots - head_slots) // 16], I16)
            nc.sync.dma_start(out=idx_rest[:], in_=idxs[:, head_slots // 16:])
            loc_rest = cpool.tile([128, (tot_slots - head_slots) // 128], BF16)
            nc.sync.dma_start(out=loc_rest[:], in_=locs[:, head_slots // 128:])
            deg_all = cpool.tile([2, NB * 128], BF16)
            nc.sync.dma_start(out=deg_all[:], in_=deg[:])

            def idx_slice(off, n):
                if off < head_slots:
                    return idx_head[:, off // 16:(off + n) // 16]
                o = off - head_slots
                return idx_rest[:, o // 16:(o + n) // 16]

            def loc_slice(off, n):
                if off < head_slots:
                    return loc_head[:, off // 128:(off + n) // 128]
                o = off - head_slots
                return loc_rest[:, o // 128:(o + n) // 128]

            # ---- constants & folded weights ----
            ident_sb = cpool.tile([128, 128], F32)
            wpre_sb = cpool.tile([cfg.DIN, cfg.DOUT], F32)
            wpost_sb = cpool.tile([cfg.DOUT, cfg.DOUT], F32)
            bpre_sb = cpool.tile([cfg.DOUT, 1], F32)
            nc.sync.dma_start(out=ident_sb[:], in_=ident_in[:])
            nc.sync.dma_start(out=wpre_sb[:], in_=wpre_in[:])
            nc.sync.dma_start(out=wpost_sb[:], in_=wpost_in[:])
            nc.sync.dma_start(out=bpre_sb[:], in_=bpre_in[:])

            wcomb_sb = cpool.tile([cfg.DIN, cfg.DOUT], BF16)
            bias2_sb = cpool.tile([2, cfg.DOUT], BF16)
            with tc.tile_pool(name="pssetup", bufs=1, space="PSUM") as pssetup:
                wpreT_ps = pssetup.tile([cfg.DOUT, cfg.DIN], F32, tag="setup")
                nc.tensor.transpose(out=wpreT_ps[:], in_=wpre_sb[:],
                                    identity=ident_sb[:])
                wpreT_sb = cpool.tile([cfg.DOUT, cfg.DIN], F32)
                nc.vector.tensor_copy(wpreT_sb[:], wpreT_ps[:])

                wcomb_ps = pssetup.tile([cfg.DIN, cfg.DOUT], F32, tag="setup")
                nc.tensor.matmul(out=wcomb_ps[:], lhsT=wpreT_sb[:],
                                 rhs=wpost_sb[:], start=True, stop=True)
                nc.vector.tensor_copy(wcomb_sb[:], wcomb_ps[:])

                bpw_ps = pssetup.tile([1, cfg.DOUT], F32, tag="setup")
                nc.tensor.matmul(out=bpw_ps[:], lhsT=bpre_sb[:], rhs=wpost_sb[:],
                                 start=True, stop=True)
                nc.vector.tensor_copy(bias2_sb[0:1, :], bpw_ps[:])
                # engine writes may not start at partition 1; DMA can
                nc.sync.dma_start(out=bias2_sb[1:2, :], in_=bposth_in[:])

            psacc = stk.enter_context(
                tc.tile_pool(name="psacc", bufs=6, space="PSUM"))
            psout = stk.enter_context(
                tc.tile_pool(name="psout", bufs=2, space="PSUM"))

            # per-block first/last (w, cu) for psum start/stop flags
            first_wc = {}
            last_wc = {}
            for b in range(NB):
                pres = [(w, cu) for w in range(NW) for cu in range(chunks[b, w])]
                first_wc[b] = pres[0]
                last_wc[b] = pres[-1]

            # ---- main loop over gather groups ----
            off = 0
            b0 = 0
            for gs in group_sizes:
                accs = [psacc.tile([128, 128], F32, name=f"acc{b0}_{i}", tag="acc")
                        for i in range(gs)]
                for w in range(NW):
                    n = int(chunks[b0:b0 + gs, w].sum()) * 128
                    C = n // 128
                    m_t = msgsp.tile([128, C, cfg.DIN], F32)
                    wb = cfg.WBASE[w]
                    nc.gpsimd.dma_gather(
                        m_t[:], data[wb:min(cfg.N, wb + 32768), :],
                        idx_slice(off, n),
                        n, n, cfg.DIN, single_packet=False)
                    mb_t = msgsbp.tile([128, C, cfg.DIN], BF16)
                    nc.scalar.copy(mb_t[:], m_t[:])
                    loc_c = loc_slice(off, n)
                    o_t = ohp.tile([128, C, 128], BF16)
                    nc.vector.tensor_tensor(
                        out=o_t[:],
                        in0=loc_c.unsqueeze(2).broadcast_to([128, C, 128]),
                        in1=iota_sb[:].unsqueeze(1).broadcast_to([128, C, 128]),
                        op=mybir.AluOpType.is_equal)
                    ch = 0
                    for bi in range(gs):
                        b = b0 + bi
                        for cu in range(chunks[b, w]):
                            nc.tensor.matmul(
                                out=accs[bi][:],
                                lhsT=mb_t[:, ch, :],
                                rhs=o_t[:, ch, :],
                                start=(w, cu) == first_wc[b],
                                stop=(w, cu) == last_wc[b])
                            ch += 1
                    off += n

                out_t = outsbp.tile([cfg.DOUT, gs * 128], BF16)
                for bi in range(gs):
                    acc_sb = accsbp.tile([128, 128], BF16)
                    nc.vector.tensor_copy(acc_sb[:], accs[bi][:])
                    outp = psout.tile([cfg.DOUT, 128], F32)
                    nc.tensor.matmul(out=outp[:], lhsT=wcomb_sb[:], rhs=acc_sb[:],
                                     start=True, stop=False)
                    nc.tensor.matmul(out=outp[:], lhsT=bias2_sb[:],
                                     rhs=deg_all[:, (b0 + bi) * 128:
                                                 (b0 + bi + 1) * 128],
                                     start=False, stop=True)
                    nc.scalar.copy(out_t[:, bi * 128:(bi + 1) * 128], outp[:])
                nc.sync.dma_start(
                    out=out[:, b0 * 128:(b0 + gs) * 128], in_=out_t[:])
                b0 += gs
    nc.compile()
    return nc


_PROGRAM_CACHE = {}


def _get_program(chunk_key, cfg=Cfg):
    key = (cfg.N, cfg.G, chunk_key)
    if key not in _PROGRAM_CACHE:
        _PROGRAM_CACHE[key] = build_program(chunk_key, cfg)
    return _PROGRAM_CACHE[key]


def make_in_maps(data, edge_src, edge_dst, W_pre, b_pre, W_post, b_post, cfg=Cfg):
    (idx_dram, loc_dram, deg_dram, chunk_key, node_core, node_slot,
     tot_slots) = preprocess(edge_src, edge_dst, cfg)
    data = np.ascontiguousarray(np.asarray(data, dtype=np.float32))
    iota = np.tile(np.arange(128, dtype=np.float32), (128, 1)).astype(NP_BF16)
    ident = np.eye(128, dtype=np.float32)
    wpre = np.asarray(W_pre, dtype=np.float32)
    wpost = np.asarray(W_post, dtype=np.float32)
    bpre = np.asarray(b_pre, dtype=np.float32).reshape(cfg.DOUT, 1)
    bpost = np.asarray(b_post, dtype=np.float32).reshape(1, cfg.DOUT)
    in_maps = []
    for c in range(cfg.NC):
        in_maps.append({
            "data": data,
            "idxs": idx_dram[c],
            "locs": loc_dram[c],
            "deg": deg_dram[c],
            "iota": iota,
            "ident": ident,
            "wpre": wpre,
            "wpost": wpost,
            "bpre": bpre,
            "bpost": bpost,
            "bposth": bpost.astype(NP_BF16),
        })
    return in_maps, chunk_key, node_core, node_slot


def assemble(outs, node_core, node_slot, cfg=Cfg):
    """outs: list of per-core [DOUT, NB*128] arrays -> [N, DOUT]."""
    stacked = np.stack([np.asarray(o) for o in outs])  # [NC, DOUT, NB*128]
    return stacked[node_core, :, node_slot].astype(np.float32)


def kernel(data, edge_src, edge_dst, W_pre, b_pre, W_post, b_post):
    cfg = Cfg
    in_maps, chunk_key, node_core, node_slot = make_in_maps(
        data, edge_src, edge_dst, W_pre, b_pre, W_post, b_post, cfg)
    nc = _get_program(chunk_key, cfg)
    res = run_bass_kernel_spmd(nc, in_maps, list(range(cfg.NC)), trace=False)
    return assemble([res.results[c]["out"] for c in range(cfg.NC)],
                    node_core, node_slot, cfg)


# revision 28
# speedup vs baseline: 1.0392x; 1.0129x over previous
"""GCN message-passing kernel for 8 trn2 NeuronCores.

Math:  out = segment_sum(h[edge_src], edge_dst) @ W_post + b_post,
       h = data @ W_pre + b_pre.
By linearity:
       out[d] = (sum_{e: dst=d} data[src_e]) @ (W_pre @ W_post)
                + deg[d] * (b_pre @ W_post) + b_post

Sharding: dst nodes are assigned to 784 bins of <=128 nodes (8 cores x 98
blocks) by a degree-balanced snake deal, so every core owns ~12500 nodes.
Each core gathers raw 512B data rows for the edges landing in its bins
(gpsimd dma_gather, int16 indices against one of 4 overlapping 32768-row
source windows), converts them to bf16 (Activation engine), segment-sums
them with one-hot bf16 matmuls on the TensorEngine (PSUM accumulation per
128-node dst block), applies the folded bf16 projection plus a rank-2 bias
term, and writes its output block-transposed ([64, 12544]); the host
scatters rows back to node order.

Window balancing: the 4 gather windows overlap (base + 32768 > next base),
so ~30% of edges may use either of two windows. A prefix-flow water-fill
per bin equalizes every (block, window) cell to <=512 edges, which makes a
uniform 4-chunks-per-cell program layout possible with only ~0.3% padding
(200832 slots/core vs 250880 for fixed 5-chunk cells).

Self-contained: only numpy + concourse imports; all shapes hardcoded.
"""

from contextlib import ExitStack

import numpy as np

import concourse.bacc as bacc
import concourse.mybir as mybir
import concourse.tile as tile
from concourse import library_config
from concourse.bass_utils import run_bass_kernel_spmd

F32 = mybir.dt.float32
BF16 = mybir.dt.bfloat16
I16 = mybir.dt.int16
NP_BF16 = mybir.dt.np(mybir.dt.bfloat16)


class Cfg:
    N = 100000          # nodes
    DIN = 128           # input features
    DOUT = 64           # output features
    NC = 8              # cores
    BS = 128            # dst block (bin) size
    NB = 98             # bins per core
    NW = 4              # src windows
    G = 6               # blocks per gather group (6 acc psum banks + 2 out)
    WBASE = (0, 22411, 44822, 67233)   # window bases (32768-row windows)


def _groups(cfg):
    sizes = []
    b = 0
    while b < cfg.NB:
        sizes.append(min(cfg.G, cfg.NB - b))
        b += cfg.G
    return sizes


def preprocess(edge_src, edge_dst, cfg=Cfg):
    """Node->bin assignment, window balancing, gather index layout."""
    src = np.asarray(edge_src).astype(np.int64)
    dst = np.asarray(edge_dst).astype(np.int64)
    E = len(src)
    N, NC, NB, NW = cfg.N, cfg.NC, cfg.NB, cfg.NW
    NBINS = NC * NB
    WBASE = np.asarray(cfg.WBASE)
    WTOP = WBASE + 32767

    # --- node -> bin: snake deal by total in-degree ---
    deg = np.bincount(dst, minlength=N)
    order = np.argsort(-deg, kind="stable")
    node_bin = np.empty(N, np.int64)
    nfull = (N + NBINS - 1) // NBINS
    for r in range(nfull):
        seg = order[r * NBINS:(r + 1) * NBINS]
        idxs = np.arange(len(seg))
        node_bin[seg] = idxs if r % 2 == 0 else (NBINS - 1 - idxs)
    # position within bin
    o2 = np.argsort(node_bin, kind="stable")
    starts = np.searchsorted(node_bin[o2], np.arange(NBINS))
    pos = np.empty(N, np.int64)
    pos[o2] = np.arange(N) - starts[node_bin[o2]]
    assert pos.max() < cfg.BS
    node_core = node_bin // NB
    node_slot = (node_bin % NB) * cfg.BS + pos

    # --- window assignment with prefix-flow balancing ---
    win = np.searchsorted(WBASE, src, side="right") - 1
    in_zone = np.full(E, -1, np.int64)
    for k in range(NW - 1):
        z = (src >= WBASE[k + 1]) & (src <= WTOP[k])
        in_zone[z] = k
        mid = (WBASE[k + 1] + WTOP[k]) // 2
        win[z] = np.where(src[z] <= mid, k, k + 1)

    ebin = node_bin[dst]
    cell = np.zeros((NBINS, NW), np.int64)
    np.add.at(cell, (ebin, win), 1)
    deg_bin = cell.sum(1)
    q, r4 = np.divmod(deg_bin, 4)
    d = np.stack([q + (r4 > 0), q + (r4 > 1), q + (r4 > 2), q], axis=1)
    F = np.cumsum(cell - d, axis=1)[:, :3]
    for k in range(NW - 1):
        f = F[:, k]
        for direction in (0, 1):
            if direction == 0:
                need = np.maximum(f, 0)
                mask = (in_zone == k) & (win == k)
            else:
                need = np.maximum(-f, 0)
                mask = (in_zone == k) & (win == k + 1)
            if need.sum() == 0:
                continue
            idx = np.where(mask)[0]
            b = ebin[idx]
            o = np.argsort(b, kind="stable")
            idx, b = idx[o], b[o]
            st = np.searchsorted(b, np.arange(NBINS))
            rank = np.arange(len(b)) - st[b]
            flip = idx[rank < need[b]]
            win[flip] = (k + 1) if direction == 0 else k

    cell = np.zeros((NBINS, NW), np.int64)
    np.add.at(cell, (ebin, win), 1)

    # --- uniform chunk table: max over cores, ceil to 128 ---
    mx = cell.reshape(NC, NB, NW).max(axis=0)
    chunks = -(-mx // 128)          # [NB, NW]
    chunks = np.maximum(chunks, 1)

    # --- slot layout: group-major, per (group, window) gather calls ---
    group_sizes = _groups(cfg)
    cell_base = np.zeros((NB, NW), np.int64)
    gather_calls = []               # (slot_base, n_slots) per (g, w)
    off = 0
    b0 = 0
    for gs in group_sizes:
        for w in range(NW):
            base = off
            for bi in range(gs):
                cell_base[b0 + bi, w] = off
                off += chunks[b0 + bi, w] * 128
            gather_calls.append((base, off - base))
        b0 += gs
    tot_slots = off
    assert tot_slots % 128 == 0

    # --- per-core slot arrays ---
    core_e = ebin // NB
    blk_e = ebin % NB
    widx = src - WBASE[win]
    assert widx.min() >= 0 and widx.max() < 32768
    loc_e = pos[dst]

    cell_id = (core_e * NB + blk_e) * NW + win
    o3 = np.argsort(cell_id, kind="stable")
    cid_s = cell_id[o3]
    counts = np.bincount(cell_id, minlength=NC * NB * NW)
    cap = (np.tile(chunks[None], (NC, 1, 1)).reshape(-1) * 128)
    assert (counts <= cap).all(), (counts.max(), "cell overflow")
    st = np.zeros(NC * NB * NW, np.int64)
    st[1:] = np.cumsum(counts)[:-1]
    rank = np.arange(E) - st[cid_s]
    slot = cell_base[blk_e[o3], win[o3]] + rank

    idx_all = np.zeros((NC, tot_slots), np.int16)
    loc_all = np.full((NC, tot_slots), -1.0, np.float32)
    idx_all[core_e[o3], slot] = widx[o3].astype(np.int16)
    loc_all[core_e[o3], slot] = loc_e[o3].astype(np.float32)

    # --- DMA wrap layouts ---
    idx_dram = np.zeros((NC, 128, tot_slots // 16), np.int16)
    loc_dram = np.zeros((NC, 128, tot_slots // 128), NP_BF16)
    for sbase, n in gather_calls:
        lin = idx_all[:, sbase:sbase + n]
        wrapped = lin.reshape(NC, n // 16, 16).transpose(0, 2, 1)
        idx_dram[:, :, sbase // 16:(sbase + n) // 16] = np.tile(wrapped, (1, 8, 1))
        ll = loc_all[:, sbase:sbase + n]
        loc_dram[:, :, sbase // 128:(sbase + n) // 128] = (
            ll.reshape(NC, n // 128, 128).transpose(0, 2, 1).astype(NP_BF16))

    # row 0: node degree; row 1: ones (rank-2 bias matmul rhs)
    deg_dram = np.ones((NC, 2, NB * 128), NP_BF16)
    flat = node_core * (NB * 128) + node_slot
    dg = np.zeros(NC * NB * 128, np.float32)
    dg[flat] = deg.astype(np.float32)
    deg_dram[:, 0, :] = dg.reshape(NC, NB * 128).astype(NP_BF16)

    chunk_key = tuple(map(int, chunks.reshape(-1)))
    return (idx_dram, loc_dram, deg_dram, chunk_key,
            node_core.astype(np.int64), node_slot.astype(np.int64), tot_slots)


def build_program(chunk_key, cfg=Cfg):
    NB, NW = cfg.NB, cfg.NW
    chunks = np.asarray(chunk_key, np.int64).reshape(NB, NW)
    group_sizes = _groups(cfg)
    tot_slots = int(chunks.sum()) * 128

    nc = bacc.Bacc("TRN2", target_bir_lowering=False, debug=True)

    data = nc.dram_tensor("data", [cfg.N, cfg.DIN], F32, kind="ExternalInput")
    idxs = nc.dram_tensor("idxs", [128, tot_slots // 16], I16, kind="ExternalInput")
    locs = nc.dram_tensor("locs", [128, tot_slots // 128], BF16, kind="ExternalInput")
    deg = nc.dram_tensor("deg", [2, NB * 128], BF16, kind="ExternalInput")
    iota_in = nc.dram_tensor("iota", [128, 128], BF16, kind="ExternalInput")
    ident_in = nc.dram_tensor("ident", [128, 128], F32, kind="ExternalInput")
    wpre_in = nc.dram_tensor("wpre", [cfg.DIN, cfg.DOUT], F32, kind="ExternalInput")
    wpost_in = nc.dram_tensor("wpost", [cfg.DOUT, cfg.DOUT], F32, kind="ExternalInput")
    bpre_in = nc.dram_tensor("bpre", [cfg.DOUT, 1], F32, kind="ExternalInput")
    bposth_in = nc.dram_tensor("bposth", [1, cfg.DOUT], BF16, kind="ExternalInput")
    out = nc.dram_tensor("out", [cfg.DOUT, NB * 128], BF16, kind="ExternalOutput")

    # slots covered by the first gather group: its idx/loc load first so
    # gathering starts while the rest of the tables stream in behind it
    head_slots = int(chunks[0:group_sizes[0], :].sum()) * 128

    with tile.TileContext(nc) as tc, ExitStack() as stk:
        nc.gpsimd.load_library(library_config.mlp)
        with (
            tc.tile_pool(name="consts", bufs=1) as cpool,
            tc.tile_pool(name="msgs", bufs=3) as msgsp,
            tc.tile_pool(name="msgsb", bufs=3) as msgsbp,
            tc.tile_pool(name="oh", bufs=3) as ohp,
            tc.tile_pool(name="accsb", bufs=3) as accsbp,
            tc.tile_pool(name="outsb", bufs=2) as outsbp,
        ):
            # ---- index/loc tables, head first ----
            iota_sb = cpool.tile([128, 128], BF16)
            idx_head = cpool.tile([128, head_slots // 16], I16)
            nc.sync.dma_start(out=idx_head[:], in_=idxs[:, :head_slots // 16])
            loc_head = cpool.tile([128, head_slots // 128], BF16)
            nc.sync.dma_start(out=loc_head[:], in_=locs[:, :head_slots // 128])
            nc.sync.dma_start(out=iota_sb[:], in_=iota_in[:])
            idx_rest = cpool.tile([128, (tot_slots - head_slots) // 16], I16)
            nc.sync.dma_start(out=idx_rest[:], in_=idxs[:, head_slots // 16:])
            loc_rest = cpool.tile([128, (tot_slots - head_slots) // 128], BF16)
            nc.sync.dma_start(out=loc_rest[:], in_=locs[:, head_slots // 128:])
            deg_all = cpool.tile([2, NB * 128], BF16)
            nc.sync.dma_start(out=deg_all[:], in_=deg[:])

            def idx_slice(off, n):
                if off < head_slots:
                    return idx_head[:, off // 16:(off + n) // 16]
                o = off - head_slots
                return idx_rest[:, o // 16:(o + n) // 16]

            def loc_slice(off, n):
                if off < head_slots:
                    return loc_head[:, off // 128:(off + n) // 128]
                o = off - head_slots
                return loc_rest[:, o // 128:(o + n) // 128]

            # ---- constants & folded weights ----
            ident_sb = cpool.tile([128, 128], F32)
            wpre_sb = cpool.tile([cfg.DIN, cfg.DOUT], F32)
            wpost_sb = cpool.tile([cfg.DOUT, cfg.DOUT], F32)
            bpre_sb = cpool.tile([cfg.DOUT, 1], F32)
            nc.sync.dma_start(out=ident_sb[:], in_=ident_in[:])
            nc.sync.dma_start(out=wpre_sb[:], in_=wpre_in[:])
            nc.sync.dma_start(out=wpost_sb[:], in_=wpost_in[:])
            nc.sync.dma_start(out=bpre_sb[:], in_=bpre_in[:])

            wcomb_sb = cpool.tile([cfg.DIN, cfg.DOUT], BF16)
            bias2_sb = cpool.tile([2, cfg.DOUT], BF16)
            with tc.tile_pool(name="pssetup", bufs=1, space="PSUM") as pssetup:
                wpreT_ps = pssetup.tile([cfg.DOUT, cfg.DIN], F32, tag="setup")
                nc.tensor.transpose(out=wpreT_ps[:], in_=wpre_sb[:],
                                    identity=ident_sb[:])
                wpreT_sb = cpool.tile([cfg.DOUT, cfg.DIN], F32)
                nc.vector.tensor_copy(wpreT_sb[:], wpreT_ps[:])

                wcomb_ps = pssetup.tile([cfg.DIN, cfg.DOUT], F32, tag="setup")
                nc.tensor.matmul(out=wcomb_ps[:], lhsT=wpreT_sb[:],
                                 rhs=wpost_sb[:], start=True, stop=True)
                nc.vector.tensor_copy(wcomb_sb[:], wcomb_ps[:])

                bpw_ps = pssetup.tile([1, cfg.DOUT], F32, tag="setup")
                nc.tensor.matmul(out=bpw_ps[:], lhsT=bpre_sb[:], rhs=wpost_sb[:],
                                 start=True, stop=True)
                nc.vector.tensor_copy(bias2_sb[0:1, :], bpw_ps[:])
                # engine writes may not start at partition 1; DMA can
                nc.sync.dma_start(out=bias2_sb[1:2, :], in_=bposth_in[:])

            psacc = stk.enter_context(
                tc.tile_pool(name="psacc", bufs=6, space="PSUM"))
            psout = stk.enter_context(
                tc.tile_pool(name="psout", bufs=2, space="PSUM"))

            # per-block first/last (w, cu) for psum start/stop flags
            first_wc = {}
            last_wc = {}
            for b in range(NB):
                pres = [(w, cu) for w in range(NW) for cu in range(chunks[b, w])]
                first_wc[b] = pres[0]
                last_wc[b] = pres[-1]

            # ---- main loop over gather groups ----
            off = 0
            b0 = 0
            for gs in group_sizes:
                accs = [psacc.tile([128, 128], F32, name=f"acc{b0}_{i}", tag="acc")
                        for i in range(gs)]
                for w in range(NW):
                    n = int(chunks[b0:b0 + gs, w].sum()) * 128
                    C = n // 128
                    m_t = msgsp.tile([128, C, cfg.DIN], F32)
                    wb = cfg.WBASE[w]
                    nc.gpsimd.dma_gather(
                        m_t[:], data[wb:min(cfg.N, wb + 32768), :],
                        idx_slice(off, n),
                        n, n, cfg.DIN, single_packet=False)
                    mb_t = msgsbp.tile([128, C, cfg.DIN], BF16)
                    nc.scalar.copy(mb_t[:], m_t[:])
                    loc_c = loc_slice(off, n)
                    o_t = ohp.tile([128, C, 128], BF16)
                    nc.vector.tensor_tensor(
                        out=o_t[:],
                        in0=loc_c.unsqueeze(2).broadcast_to([128, C, 128]),
                        in1=iota_sb[:].unsqueeze(1).broadcast_to([128, C, 128]),
                        op=mybir.AluOpType.is_equal)
                    ch = 0
                    for bi in range(gs):
                        b = b0 + bi
                        for cu in range(chunks[b, w]):
                            nc.tensor.matmul(
                                out=accs[bi][:],
                                lhsT=mb_t[:, ch, :],
                                rhs=o_t[:, ch, :],
                                start=(w, cu) == first_wc[b],
                                stop=(w, cu) == last_wc[b])
                            ch += 1
                    off += n

                out_t = outsbp.tile([cfg.DOUT, gs * 128], BF16)
                for bi in range(gs):
                    acc_sb = accsbp.tile([128, 128], BF16)
                    nc.vector.tensor_copy(acc_sb[:], accs[bi][:])
                    outp = psout.tile([cfg.DOUT, 128], F32)
                    nc.tensor.matmul(out=outp[:], lhsT=wcomb_sb[:], rhs=acc_sb[:],
                                     start=True, stop=False)
                    nc.tensor.matmul(out=outp[:], lhsT=bias2_sb[:],
                                     rhs=deg_all[:, (b0 + bi) * 128:
                                                 (b0 + bi + 1) * 128],
                                     start=False, stop=True)
                    nc.scalar.copy(out_t[:, bi * 128:(bi + 1) * 128], outp[:])
                nc.sync.dma_start(
                    out=out[:, b0 * 128:(b0 + gs) * 128], in_=out_t[:])
                b0 += gs
    nc.compile()
    return nc


_PROGRAM_CACHE = {}


def _get_program(chunk_key, cfg=Cfg):
    key = (cfg.N, cfg.G, chunk_key)
    if key not in _PROGRAM_CACHE:
        _PROGRAM_CACHE[key] = build_program(chunk_key, cfg)
    return _PROGRAM_CACHE[key]


def make_in_maps(data, edge_src, edge_dst, W_pre, b_pre, W_post, b_post, cfg=Cfg):
    (idx_dram, loc_dram, deg_dram, chunk_key, node_core, node_slot,
     tot_slots) = preprocess(edge_src, edge_dst, cfg)
    data = np.ascontiguousarray(np.asarray(data, dtype=np.float32))
    iota = np.tile(np.arange(128, dtype=np.float32), (128, 1)).astype(NP_BF16)
    ident = np.eye(128, dtype=np.float32)
    wpre = np.asarray(W_pre, dtype=np.float32)
    wpost = np.asarray(W_post, dtype=np.float32)
    bpre = np.asarray(b_pre, dtype=np.float32).reshape(cfg.DOUT, 1)
    bpost = np.asarray(b_post, dtype=np.float32).reshape(1, cfg.DOUT)
    in_maps = []
    for c in range(cfg.NC):
        in_maps.append({
            "data": data,
            "idxs": idx_dram[c],
            "locs": loc_dram[c],
            "deg": deg_dram[c],
            "iota": iota,
            "ident": ident,
            "wpre": wpre,
            "wpost": wpost,
            "bpre": bpre,
            "bpost": bpost,
            "bposth": bpost.astype(NP_BF16),
        })
    return in_maps, chunk_key, node_core, node_slot


def assemble(outs, node_core, node_slot, cfg=Cfg):
    """outs: list of per-core [DOUT, NB*128] arrays -> [N, DOUT]."""
    stacked = np.stack([np.asarray(o) for o in outs])  # [NC, DOUT, NB*128]
    return stacked[node_core, :, node_slot].astype(np.float32)


def kernel(data, edge_src, edge_dst, W_pre, b_pre, W_post, b_post):
    cfg = Cfg
    in_maps, chunk_key, node_core, node_slot = make_in_maps(
        data, edge_src, edge_dst, W_pre, b_pre, W_post, b_post, cfg)
    nc = _get_program(chunk_key, cfg)
    res = run_bass_kernel_spmd(nc, in_maps, list(range(cfg.NC)), trace=False)
    return assemble([res.results[c]["out"] for c in range(cfg.NC)],
                    node_core, node_slot, cfg)
